# revision 24
# baseline (speedup 1.0000x reference)
"""Trainium2 Bass kernel for nn_LstmClassifier: batch-sharded LSTM over 8 cores.

Reference math (per batch row):
    h0 = relu(x @ W_in.T + b_in); c0 = 0
    64 steps of: gates = h @ (W_ih + W_hh).T + (b_ih + b_hh)   # input == hidden
                 i,f,g,o = split(gates); c = sig(f)*c + sig(i)*tanh(g); h = sig(o)*tanh(c)
    out[:, t, :] = h_t @ W_out.T + b_out

Device layout is hidden-major ("transposed"): h.T packed as [128 part, 4, 256],
so gate-unit biases are per-partition and fuse into the activation instructions,
and no transposes are needed anywhere in the recurrence.
"""
import sys
import types
import numpy as np

sys.path.insert(0, "/opt/trn_rl_repo")

import concourse.bass as bass  # noqa: E402
import concourse.tile as tile  # noqa: E402
from concourse import bacc, mybir  # noqa: E402
from concourse.bass_utils import run_bass_kernel_spmd  # noqa: E402

B, IN_DIM, HID, OUT_DIM, T = 2048, 1024, 512, 256, 64
NCORES = 8
BSH = B // NCORES          # 256 batch rows per core
KH = HID // 128            # 4 hidden k-tiles
KI = IN_DIM // 128         # 8 input k-tiles
NMT = 4 * HID // 128       # 16 gate m-tiles
F32 = mybir.dt.float32
F16 = mybir.dt.float16
AF = mybir.ActivationFunctionType

LAST_EXEC_NS = None


def _install_ntff_hook():
    try:
        import antenv.axon_hooks  # noqa: F401
        return True
    except ImportError:
        pass
    try:
        if "/root/.axon_site" not in sys.path:
            sys.path.insert(0, "/root/.axon_site")
        from trn_agent_boot.trn_boot import _ntff_profile_via_ctypes
        hook = _ntff_profile_via_ctypes("/opt/axon/libaxon_pjrt.so")
        if hook is None:
            return False
        import antenv
        mod = types.ModuleType("antenv.axon_hooks")
        mod._hook = hook
        mod.get_axon_ntff_profile_hook = lambda: mod._hook
        mod.set_axon_ntff_profile_hook = lambda h: setattr(mod, "_hook", h)
        antenv.axon_hooks = mod
        sys.modules["antenv.axon_hooks"] = mod
        return True
    except Exception:
        return False


def build_program(steps=T):
    nc = bacc.Bacc("TRN2", target_bir_lowering=False, debug=False)

    xT_d = nc.dram_tensor("xT", [128, KI, BSH], F16, kind="ExternalInput").ap()
    winT_d = nc.dram_tensor("winT", [128, KI, HID], F16, kind="ExternalInput").ap()
    wcT_d = nc.dram_tensor("wcT", [128, KH, 4 * HID], F16, kind="ExternalInput").ap()
    woutT_d = nc.dram_tensor("woutT", [128, KH, OUT_DIM], F16, kind="ExternalInput").ap()
    bin_d = nc.dram_tensor("bin", [128, KH], F32, kind="ExternalInput").ap()
    bc_d = nc.dram_tensor("bc", [128, NMT], F32, kind="ExternalInput").ap()
    bout_d = nc.dram_tensor("bout", [128, OUT_DIM], F32, kind="ExternalInput").ap()
    out_d = nc.dram_tensor("out", [BSH, steps, OUT_DIM], F32, kind="ExternalOutput").ap()
    # [p, m, t, o]: batch row = m*128 + p
    out_v = out_d.rearrange("(m p) t o -> p m t o", p=128)

    from contextlib import ExitStack
    with tile.TileContext(nc) as tc, ExitStack() as ctx:
        consts = ctx.enter_context(tc.tile_pool(name="consts", bufs=1))
        hpool = ctx.enter_context(tc.tile_pool(name="h", bufs=4))
        cpool = ctx.enter_context(tc.tile_pool(name="c", bufs=3))
        actp = ctx.enter_context(tc.tile_pool(name="acts", bufs=6))
        dvep = ctx.enter_context(tc.tile_pool(name="dvet", bufs=4))
        osbp = ctx.enter_context(tc.tile_pool(name="osb", bufs=4))
        pg_pool = ctx.enter_context(tc.tile_pool(name="pgates", bufs=3, space="PSUM"))
        po_pool = ctx.enter_context(tc.tile_pool(name="pout", bufs=2, space="PSUM"))

        # ---- load constants ----
        xT = consts.tile([128, KI, BSH], F16)
        nc.sync.dma_start(xT[:], xT_d[:])
        winT = consts.tile([128, KI, HID], F16)
        nc.sync.dma_start(winT[:], winT_d[:])
        wcT = consts.tile([128, KH, 4 * HID], F16)
        nc.sync.dma_start(wcT[:], wcT_d[:])
        woutT = consts.tile([128, KH, OUT_DIM], F16)
        nc.sync.dma_start(woutT[:], woutT_d[:])
        bin_sb = consts.tile([128, KH], F32)
        nc.sync.dma_start(bin_sb[:], bin_d[:])
        bc_sb = consts.tile([128, NMT], F32)
        nc.sync.dma_start(bc_sb[:], bc_d[:])
        bout_sb = consts.tile([128, OUT_DIM], F32)
        nc.sync.dma_start(bout_sb[:], bout_d[:])

        # ---- input projection: h0.T = relu(W_in @ x.T + b_in) ----
        h = hpool.tile([128, KH, BSH], F16, tag="h")
        ph0 = pg_pool.tile([128, 4, BSH], F32, tag="pg")
        for m in range(KH):
            for k in range(KI):
                nc.tensor.matmul(
                    ph0[:, m, :],
                    winT[:, k, m * 128:(m + 1) * 128],
                    xT[:, k, :],
                    start=(k == 0 and m % 2 == 0),
                    stop=(k == KI - 1 and m % 2 == 1),
                )
        for m in range(KH):
            nc.scalar.activation(
                h[:, m, :], ph0[:, m, :], AF.Relu, bias=bin_sb[:, m:m + 1]
            )

        bout_b = bout_sb[:].unsqueeze(1).broadcast_to([128, 2, OUT_DIM])

        def emit_outproj_mms(po, h_src, ks):
            # po is one PSUM bank: a single accumulation group (start on first
            # matmul touching the bank, stop on the last; unwritten bytes of a
            # started zero-region read as 0, so m=1's k=0 matmul may accumulate)
            for k in ks:
                for m in range(2):
                    nc.tensor.matmul(
                        po[:, m, :],
                        h_src[:, k, m * 128:(m + 1) * 128],
                        woutT[:, k, :],
                        start=(k == 0 and m == 0),
                        stop=(k == KH - 1 and m == 1),
                    )

        def emit_outproj_tail(po, t_idx):
            osb = osbp.tile([128, 2, OUT_DIM], F32, tag="osb", name=f"osb{t_idx}")
            nc.vector.tensor_add(osb[:], po[:], bout_b)
            nc.sync.dma_start(out_v[:, :, t_idx, :], osb[:])

        def emit_outproj(h_src, t_idx):
            po = po_pool.tile([128, 2, OUT_DIM], F32, tag="po", name=f"po{t_idx}")
            emit_outproj_mms(po, h_src, range(KH))
            emit_outproj_tail(po, t_idx)

        c = None
        for t in range(steps):
            h_new = hpool.tile([128, KH, BSH], F16, tag="h")
            c_new = cpool.tile([128, KH, BSH], F16, tag="c")
            wave = [None] * KH  # per wave: (sig_i, sig_f, sig_o, tng)
            pgs = [None] * KH

            def mm_block(j, ks):
                # pg spans 2 banks (gi 0,1 | gi 2,3): one accumulation group
                # per bank — start on the bank's first matmul, stop on its last
                for k in ks:
                    for gi in range(4):
                        mt = 4 * j + gi
                        nc.tensor.matmul(
                            pgs[j][:, gi, :],
                            wcT[:, k, mt * 128:(mt + 1) * 128],
                            h[:, k, :],
                            start=(k == 0 and gi % 2 == 0),
                            stop=(k == KH - 1 and gi % 2 == 1),
                        )

            def act_block(j):
                pg = pgs[j]
                sig_i = actp.tile([128, BSH], F16, tag="sig_i", name=f"si{t}_{j}")
                sig_f = actp.tile([128, BSH], F16, tag="sig_f", name=f"sf{t}_{j}")
                sig_o = actp.tile([128, BSH], F16, tag="sig_o", name=f"so{t}_{j}")
                tng = actp.tile([128, BSH], F16, tag="tng", name=f"tg{t}_{j}")
                nc.scalar.activation(sig_i[:], pg[:, 0, :], AF.Sigmoid, bias=bc_sb[:, 4 * j + 0:4 * j + 1])
                nc.scalar.activation(sig_f[:], pg[:, 1, :], AF.Sigmoid, bias=bc_sb[:, 4 * j + 1:4 * j + 2])
                nc.scalar.activation(tng[:], pg[:, 3, :], AF.Tanh, bias=bc_sb[:, 4 * j + 3:4 * j + 4])
                nc.scalar.activation(sig_o[:], pg[:, 2, :], AF.Sigmoid, bias=bc_sb[:, 4 * j + 2:4 * j + 3])
                wave[j] = (sig_i, sig_f, sig_o, tng)

            def cell_update(j):
                # c_new[j] = sig(f)*c[j] + sig(i)*tanh(g)
                sig_i, sig_f, sig_o, tng = wave[j]
                if t == 0:
                    # c0 == 0: c1 = sig(i) * tanh(g)
                    nc.vector.tensor_mul(c_new[:, j, :], sig_i[:], tng[:])
                else:
                    t1 = dvep.tile([128, BSH], F16, tag="t1", name=f"t1_{t}_{j}")
                    nc.vector.tensor_mul(t1[:], sig_i[:], tng[:])
                    nc.vector.tensor_mul(c_new[:, j, :], sig_f[:], c[:, j, :])
                    nc.vector.tensor_add(c_new[:, j, :], c_new[:, j, :], t1[:])

            def h_update(jpair):
                # tanh(c) merged over a wave pair, then h = sig(o) * tanh(c)
                tnc = dvep.tile([128, 2, BSH], F16, tag="tnc", name=f"tnc{t}_{jpair}")
                nc.scalar.activation(tnc[:], c_new[:, 2 * jpair:2 * jpair + 2, :], AF.Tanh)
                for j in (2 * jpair, 2 * jpair + 1):
                    nc.vector.tensor_mul(h_new[:, j, :], wave[j][2][:], tnc[:, j - 2 * jpair, :])

            def h_update_single(j):
                # last waves get individual tanh(c) so h[2] exits the
                # end-of-step dependency ring before h[3]'s chain completes
                tnc = dvep.tile([128, 2, BSH], F16, tag="tnc", name=f"tncs{t}_{j}")
                nc.scalar.activation(tnc[:, 0, :], c_new[:, j, :], AF.Tanh)
                nc.vector.tensor_mul(h_new[:, j, :], wave[j][2][:], tnc[:, 0, :])

            # PE stream phased globally by k: the engine executes in order, so
            # all k0/k1 matmuls (early h slices) of wave0/outproj/wave1/wave2
            # issue first (~3us of work overlapping the end-of-step ring), then
            # the k2 blocks, then the k3 blocks gated on the final h slice.
            pgs[0] = pg_pool.tile([128, 4, BSH], F32, tag="pg", name=f"pg{t}_0")
            mm_block(0, [0, 1])
            po = None
            if t >= 1:
                po = po_pool.tile([128, 2, OUT_DIM], F32, tag="po", name=f"po{t-1}")
                emit_outproj_mms(po, h, [0, 1])
            pgs[1] = pg_pool.tile([128, 4, BSH], F32, tag="pg", name=f"pg{t}_1")
            mm_block(1, [0, 1])
            pgs[2] = pg_pool.tile([128, 4, BSH], F32, tag="pg", name=f"pg{t}_2")
            mm_block(2, [0, 1])
            mm_block(0, [2])
            if t >= 1:
                emit_outproj_mms(po, h, [2])
            mm_block(1, [2])
            mm_block(2, [2])
            mm_block(0, [3])
            act_block(0)
            mm_block(1, [3])
            act_block(1)
            cell_update(0)
            if t >= 1:
                emit_outproj_mms(po, h, [3])
                emit_outproj_tail(po, t - 1)
            mm_block(2, [3])
            act_block(2)
            cell_update(1)
            h_update(0)
            pgs[3] = pg_pool.tile([128, 4, BSH], F32, tag="pg", name=f"pg{t}_3")
            mm_block(3, range(KH))
            act_block(3)
            cell_update(2)
            h_update_single(2)
            cell_update(KH - 1)
            h_update_single(3)

            h = h_new
            c = c_new
        emit_outproj(h, steps - 1)

    nc.compile()
    return nc


_PROGRAM = None


def _get_program():
    global _PROGRAM
    if _PROGRAM is None:
        _PROGRAM = build_program()
    return _PROGRAM


def _pack_inputs(x, W_in, b_in, W_ih, b_ih, W_hh, b_hh, W_out, b_out):
    f16, f32 = np.float16, np.float32
    Wc = (np.asarray(W_ih, f32) + np.asarray(W_hh, f32))
    bc = (np.asarray(b_ih, f32) + np.asarray(b_hh, f32))
    # reorder gate rows to m-tiles [i_j, f_j, o_j, g_j] (PyTorch order i,f,g,o)
    base = {0: 0, 1: HID, 2: 3 * HID, 3: 2 * HID}  # gi -> original row block
    perm = np.concatenate([
        np.arange(base[gi] + j * 128, base[gi] + j * 128 + 128)
        for j in range(KH) for gi in range(4)
    ])
    Wc_r = Wc[perm]                      # [2048, 512]
    bc_r = bc[perm]                      # [2048]
    wcT = np.ascontiguousarray(
        Wc_r.T.reshape(KH, 128, 4 * HID).transpose(1, 0, 2).astype(f16))
    winT = np.ascontiguousarray(
        np.asarray(W_in, f32).T.reshape(KI, 128, HID).transpose(1, 0, 2).astype(f16))
    woutT = np.ascontiguousarray(
        np.asarray(W_out, f32).T.reshape(KH, 128, OUT_DIM).transpose(1, 0, 2).astype(f16))
    bin_p = np.ascontiguousarray(np.asarray(b_in, f32).reshape(KH, 128).T)
    bc_p = np.ascontiguousarray(bc_r.reshape(NMT, 128).T)
    bout_p = np.ascontiguousarray(np.broadcast_to(np.asarray(b_out, f32), (128, OUT_DIM)))

    shared = {
        "winT": winT, "wcT": wcT, "woutT": woutT,
        "bin": bin_p, "bc": bc_p, "bout": bout_p,
    }
    in_maps = []
    x = np.asarray(x, f32)
    for cid in range(NCORES):
        xs = x[cid * BSH:(cid + 1) * BSH]          # [256, 1024]
        xT = np.ascontiguousarray(
            xs.T.reshape(KI, 128, BSH).transpose(1, 0, 2).astype(f16))
        in_maps.append({"xT": xT, **shared})
    return in_maps


def kernel(x, W_in, b_in, W_ih, b_ih, W_hh, b_hh, W_out, b_out, trace=False):
    global LAST_EXEC_NS
    nc = _get_program()
    in_maps = _pack_inputs(x, W_in, b_in, W_ih, b_ih, W_hh, b_hh, W_out, b_out)
    if trace:
        trace = _install_ntff_hook()
    res = run_bass_kernel_spmd(nc, in_maps, core_ids=list(range(NCORES)), trace=trace)
    LAST_EXEC_NS = res.exec_time_ns
    return np.concatenate([res.results[c]["out"] for c in range(NCORES)], axis=0)


# revision 25
# speedup vs baseline: 1.0908x; 1.0908x over previous
"""Trainium2 Bass kernel for nn_LstmClassifier: batch-sharded LSTM over 8 cores.

Reference math (per batch row):
    h0 = relu(x @ W_in.T + b_in); c0 = 0
    64 steps of: gates = h @ (W_ih + W_hh).T + (b_ih + b_hh)   # input == hidden
                 i,f,g,o = split(gates); c = sig(f)*c + sig(i)*tanh(g); h = sig(o)*tanh(c)
    out[:, t, :] = h_t @ W_out.T + b_out

Device layout is hidden-major ("transposed"): h.T packed as [128 part, 4, 256],
so gate-unit biases are per-partition and fuse into the activation instructions,
and no transposes are needed anywhere in the recurrence.
"""
import sys
import types
import numpy as np

sys.path.insert(0, "/opt/trn_rl_repo")

import concourse.bass as bass  # noqa: E402
import concourse.tile as tile  # noqa: E402
from concourse import bacc, mybir  # noqa: E402
from concourse.bass_utils import run_bass_kernel_spmd  # noqa: E402

B, IN_DIM, HID, OUT_DIM, T = 2048, 1024, 512, 256, 64
NCORES = 8
BSH = B // NCORES          # 256 batch rows per core
KH = HID // 128            # 4 hidden k-tiles
KI = IN_DIM // 128         # 8 input k-tiles
NMT = 4 * HID // 128       # 16 gate m-tiles
F32 = mybir.dt.float32
F16 = mybir.dt.float16
AF = mybir.ActivationFunctionType

LAST_EXEC_NS = None


def _install_ntff_hook():
    try:
        import antenv.axon_hooks  # noqa: F401
        return True
    except ImportError:
        pass
    try:
        if "/root/.axon_site" not in sys.path:
            sys.path.insert(0, "/root/.axon_site")
        from trn_agent_boot.trn_boot import _ntff_profile_via_ctypes
        hook = _ntff_profile_via_ctypes("/opt/axon/libaxon_pjrt.so")
        if hook is None:
            return False
        import antenv
        mod = types.ModuleType("antenv.axon_hooks")
        mod._hook = hook
        mod.get_axon_ntff_profile_hook = lambda: mod._hook
        mod.set_axon_ntff_profile_hook = lambda h: setattr(mod, "_hook", h)
        antenv.axon_hooks = mod
        sys.modules["antenv.axon_hooks"] = mod
        return True
    except Exception:
        return False


def build_program(steps=T):
    nc = bacc.Bacc("TRN2", target_bir_lowering=False, debug=False)

    xT_d = nc.dram_tensor("xT", [128, KI, BSH], F16, kind="ExternalInput").ap()
    winT_d = nc.dram_tensor("winT", [128, KI, HID], F16, kind="ExternalInput").ap()
    wcT_d = nc.dram_tensor("wcT", [128, KH, 4 * HID], F16, kind="ExternalInput").ap()
    woutT_d = nc.dram_tensor("woutT", [128, KH, OUT_DIM], F16, kind="ExternalInput").ap()
    bin_d = nc.dram_tensor("bin", [128, KH], F32, kind="ExternalInput").ap()
    bc_d = nc.dram_tensor("bc", [128, NMT], F32, kind="ExternalInput").ap()
    bout_d = nc.dram_tensor("bout", [128, OUT_DIM], F32, kind="ExternalInput").ap()
    out_d = nc.dram_tensor("out", [BSH, steps, OUT_DIM], F32, kind="ExternalOutput").ap()
    # [p, m, t, o]: batch row = m*128 + p
    out_v = out_d.rearrange("(m p) t o -> p m t o", p=128)

    from contextlib import ExitStack
    with tile.TileContext(nc) as tc, ExitStack() as ctx:
        consts = ctx.enter_context(tc.tile_pool(name="consts", bufs=1))
        hpool = ctx.enter_context(tc.tile_pool(name="h", bufs=4))
        cpool = ctx.enter_context(tc.tile_pool(name="c", bufs=3))
        actp = ctx.enter_context(tc.tile_pool(name="acts", bufs=6))
        dvep = ctx.enter_context(tc.tile_pool(name="dvet", bufs=4))
        osbp = ctx.enter_context(tc.tile_pool(name="osb", bufs=4))
        pg_pool = ctx.enter_context(tc.tile_pool(name="pgates", bufs=3, space="PSUM"))
        po_pool = ctx.enter_context(tc.tile_pool(name="pout", bufs=2, space="PSUM"))

        # ---- load constants ----
        xT = consts.tile([128, KI, BSH], F16)
        nc.sync.dma_start(xT[:], xT_d[:])
        winT = consts.tile([128, KI, HID], F16)
        nc.sync.dma_start(winT[:], winT_d[:])
        wcT = consts.tile([128, KH, 4 * HID], F16)
        nc.sync.dma_start(wcT[:], wcT_d[:])
        woutT = consts.tile([128, KH, OUT_DIM], F16)
        nc.sync.dma_start(woutT[:], woutT_d[:])
        bin_sb = consts.tile([128, KH], F32)
        nc.sync.dma_start(bin_sb[:], bin_d[:])
        bc_sb = consts.tile([128, NMT], F32)
        nc.sync.dma_start(bc_sb[:], bc_d[:])
        bout_sb = consts.tile([128, OUT_DIM], F32)
        nc.sync.dma_start(bout_sb[:], bout_d[:])

        # ---- input projection: h0.T = relu(W_in @ x.T + b_in) ----
        h = hpool.tile([128, KH, BSH], F16, tag="h")
        ph0 = pg_pool.tile([128, 4, BSH], F32, tag="pg")
        for m in range(KH):
            for k in range(KI):
                nc.tensor.matmul(
                    ph0[:, m, :],
                    winT[:, k, m * 128:(m + 1) * 128],
                    xT[:, k, :],
                    start=(k == 0 and m % 2 == 0),
                    stop=(k == KI - 1 and m % 2 == 1),
                )
        for m in range(KH):
            nc.scalar.activation(
                h[:, m, :], ph0[:, m, :], AF.Relu, bias=bin_sb[:, m:m + 1]
            )

        bout_b = bout_sb[:].unsqueeze(1).broadcast_to([128, 2, OUT_DIM])

        def emit_outproj_mms(po, h_src, ks):
            # po is one PSUM bank: a single accumulation group (start on first
            # matmul touching the bank, stop on the last; unwritten bytes of a
            # started zero-region read as 0, so m=1's k=0 matmul may accumulate)
            for k in ks:
                for m in range(2):
                    nc.tensor.matmul(
                        po[:, m, :],
                        h_src[:, k, m * 128:(m + 1) * 128],
                        woutT[:, k, :],
                        start=(k == 0 and m == 0),
                        stop=(k == KH - 1 and m == 1),
                    )

        def emit_outproj_tail(po, t_idx):
            osb = osbp.tile([128, 2, OUT_DIM], F32, tag="osb", name=f"osb{t_idx}")
            nc.vector.tensor_add(osb[:], po[:], bout_b)
            nc.sync.dma_start(out_v[:, :, t_idx, :], osb[:])

        def emit_outproj(h_src, t_idx):
            po = po_pool.tile([128, 2, OUT_DIM], F32, tag="po", name=f"po{t_idx}")
            emit_outproj_mms(po, h_src, range(KH))
            emit_outproj_tail(po, t_idx)

        c = None
        for t in range(steps):
            h_new = hpool.tile([128, KH, BSH], F16, tag="h")
            c_new = cpool.tile([128, KH, BSH], F16, tag="c")
            wave = [None] * KH  # per wave: (sig_i, sig_f, sig_o, tng)
            pgs = [None] * KH

            def mm_block(j, ks):
                # pg spans 2 banks (gi 0,1 | gi 2,3): one accumulation group
                # per bank — start on the bank's first matmul, stop on its last
                for k in ks:
                    for gi in range(4):
                        mt = 4 * j + gi
                        nc.tensor.matmul(
                            pgs[j][:, gi, :],
                            wcT[:, k, mt * 128:(mt + 1) * 128],
                            h[:, k, :],
                            start=(k == 0 and gi % 2 == 0),
                            stop=(k == KH - 1 and gi % 2 == 1),
                        )

            def act_block(j):
                pg = pgs[j]
                sig_i = actp.tile([128, BSH], F16, tag="sig_i", name=f"si{t}_{j}")
                sig_f = actp.tile([128, BSH], F16, tag="sig_f", name=f"sf{t}_{j}")
                sig_o = actp.tile([128, BSH], F16, tag="sig_o", name=f"so{t}_{j}")
                tng = actp.tile([128, BSH], F16, tag="tng", name=f"tg{t}_{j}")
                nc.scalar.activation(sig_i[:], pg[:, 0, :], AF.Sigmoid, bias=bc_sb[:, 4 * j + 0:4 * j + 1])
                nc.scalar.activation(sig_f[:], pg[:, 1, :], AF.Sigmoid, bias=bc_sb[:, 4 * j + 1:4 * j + 2])
                nc.scalar.activation(tng[:], pg[:, 3, :], AF.Tanh, bias=bc_sb[:, 4 * j + 3:4 * j + 4])
                nc.scalar.activation(sig_o[:], pg[:, 2, :], AF.Sigmoid, bias=bc_sb[:, 4 * j + 2:4 * j + 3])
                wave[j] = (sig_i, sig_f, sig_o, tng)

            def cell_update(j):
                # c_new[j] = sig(f)*c[j] + sig(i)*tanh(g)
                sig_i, sig_f, sig_o, tng = wave[j]
                if t == 0:
                    # c0 == 0: c1 = sig(i) * tanh(g)
                    nc.vector.tensor_mul(c_new[:, j, :], sig_i[:], tng[:])
                else:
                    t1 = dvep.tile([128, BSH], F16, tag="t1", name=f"t1_{t}_{j}")
                    nc.vector.tensor_mul(t1[:], sig_i[:], tng[:])
                    nc.vector.tensor_mul(c_new[:, j, :], sig_f[:], c[:, j, :])
                    nc.vector.tensor_add(c_new[:, j, :], c_new[:, j, :], t1[:])

            def h_update(jpair):
                # tanh(c) merged over a wave pair, then h = sig(o) * tanh(c)
                tnc = dvep.tile([128, 2, BSH], F16, tag="tnc", name=f"tnc{t}_{jpair}")
                nc.scalar.activation(tnc[:], c_new[:, 2 * jpair:2 * jpair + 2, :], AF.Tanh)
                for j in (2 * jpair, 2 * jpair + 1):
                    nc.vector.tensor_mul(h_new[:, j, :], wave[j][2][:], tnc[:, j - 2 * jpair, :])

            def h_update_single(j):
                # last waves get individual tanh(c) so h[2] exits the
                # end-of-step dependency ring before h[3]'s chain completes
                tnc = dvep.tile([128, 2, BSH], F16, tag="tnc", name=f"tncs{t}_{j}")
                nc.scalar.activation(tnc[:, 0, :], c_new[:, j, :], AF.Tanh)
                nc.vector.tensor_mul(h_new[:, j, :], wave[j][2][:], tnc[:, 0, :])

            # PE stream: front-load k0-k2 matmuls of wave0/outproj/wave1 (they
            # need only early h slices) so the PE advances through the
            # end-of-step ring while the last wave's pointwise chain finishes;
            # the k3 blocks (gated on the final h slice) come right after.
            pgs[0] = pg_pool.tile([128, 4, BSH], F32, tag="pg", name=f"pg{t}_0")
            mm_block(0, range(KH - 1))
            po = None
            if t >= 1:
                po = po_pool.tile([128, 2, OUT_DIM], F32, tag="po", name=f"po{t-1}")
                emit_outproj_mms(po, h, range(KH - 1))
            pgs[1] = pg_pool.tile([128, 4, BSH], F32, tag="pg", name=f"pg{t}_1")
            mm_block(1, range(KH - 1))
            mm_block(0, [KH - 1])
            act_block(0)
            mm_block(1, [KH - 1])
            act_block(1)
            cell_update(0)
            if t >= 1:
                emit_outproj_mms(po, h, [KH - 1])
                emit_outproj_tail(po, t - 1)
            pgs[2] = pg_pool.tile([128, 4, BSH], F32, tag="pg", name=f"pg{t}_2")
            mm_block(2, range(KH))
            act_block(2)
            cell_update(1)
            h_update(0)
            pgs[3] = pg_pool.tile([128, 4, BSH], F32, tag="pg", name=f"pg{t}_3")
            mm_block(3, range(KH))
            act_block(3)
            cell_update(2)
            h_update_single(2)
            cell_update(KH - 1)
            h_update_single(3)

            h = h_new
            c = c_new
        emit_outproj(h, steps - 1)

    nc.compile()
    return nc


_PROGRAM = None


def _get_program():
    global _PROGRAM
    if _PROGRAM is None:
        _PROGRAM = build_program()
    return _PROGRAM


def _pack_inputs(x, W_in, b_in, W_ih, b_ih, W_hh, b_hh, W_out, b_out):
    f16, f32 = np.float16, np.float32
    Wc = (np.asarray(W_ih, f32) + np.asarray(W_hh, f32))
    bc = (np.asarray(b_ih, f32) + np.asarray(b_hh, f32))
    # reorder gate rows to m-tiles [i_j, f_j, o_j, g_j] (PyTorch order i,f,g,o)
    base = {0: 0, 1: HID, 2: 3 * HID, 3: 2 * HID}  # gi -> original row block
    perm = np.concatenate([
        np.arange(base[gi] + j * 128, base[gi] + j * 128 + 128)
        for j in range(KH) for gi in range(4)
    ])
    Wc_r = Wc[perm]                      # [2048, 512]
    bc_r = bc[perm]                      # [2048]
    wcT = np.ascontiguousarray(
        Wc_r.T.reshape(KH, 128, 4 * HID).transpose(1, 0, 2).astype(f16))
    winT = np.ascontiguousarray(
        np.asarray(W_in, f32).T.reshape(KI, 128, HID).transpose(1, 0, 2).astype(f16))
    woutT = np.ascontiguousarray(
        np.asarray(W_out, f32).T.reshape(KH, 128, OUT_DIM).transpose(1, 0, 2).astype(f16))
    bin_p = np.ascontiguousarray(np.asarray(b_in, f32).reshape(KH, 128).T)
    bc_p = np.ascontiguousarray(bc_r.reshape(NMT, 128).T)
    bout_p = np.ascontiguousarray(np.broadcast_to(np.asarray(b_out, f32), (128, OUT_DIM)))

    shared = {
        "winT": winT, "wcT": wcT, "woutT": woutT,
        "bin": bin_p, "bc": bc_p, "bout": bout_p,
    }
    in_maps = []
    x = np.asarray(x, f32)
    for cid in range(NCORES):
        xs = x[cid * BSH:(cid + 1) * BSH]          # [256, 1024]
        xT = np.ascontiguousarray(
            xs.T.reshape(KI, 128, BSH).transpose(1, 0, 2).astype(f16))
        in_maps.append({"xT": xT, **shared})
    return in_maps


def kernel(x, W_in, b_in, W_ih, b_ih, W_hh, b_hh, W_out, b_out, trace=False):
    global LAST_EXEC_NS
    nc = _get_program()
    in_maps = _pack_inputs(x, W_in, b_in, W_ih, b_ih, W_hh, b_hh, W_out, b_out)
    if trace:
        trace = _install_ntff_hook()
    res = run_bass_kernel_spmd(nc, in_maps, core_ids=list(range(NCORES)), trace=trace)
    LAST_EXEC_NS = res.exec_time_ns
    return np.concatenate([res.results[c]["out"] for c in range(NCORES)], axis=0)


# revision 26
# speedup vs baseline: 1.0958x; 1.0045x over previous
"""Trainium2 Bass kernel for nn_LstmClassifier: batch-sharded LSTM over 8 cores.

Reference math (per batch row):
    h0 = relu(x @ W_in.T + b_in); c0 = 0
    64 steps of: gates = h @ (W_ih + W_hh).T + (b_ih + b_hh)   # input == hidden
                 i,f,g,o = split(gates); c = sig(f)*c + sig(i)*tanh(g); h = sig(o)*tanh(c)
    out[:, t, :] = h_t @ W_out.T + b_out

Device layout is hidden-major ("transposed"): h.T packed as [128 part, 4, 256],
so gate-unit biases are per-partition and fuse into the activation instructions,
and no transposes are needed anywhere in the recurrence.
"""
import sys
import types
import numpy as np

sys.path.insert(0, "/opt/trn_rl_repo")

import concourse.bass as bass  # noqa: E402
import concourse.tile as tile  # noqa: E402
from concourse import bacc, mybir  # noqa: E402
from concourse.bass_utils import run_bass_kernel_spmd  # noqa: E402

B, IN_DIM, HID, OUT_DIM, T = 2048, 1024, 512, 256, 64
NCORES = 8
BSH = B // NCORES          # 256 batch rows per core
KH = HID // 128            # 4 hidden k-tiles
KI = IN_DIM // 128         # 8 input k-tiles
NMT = 4 * HID // 128       # 16 gate m-tiles
F32 = mybir.dt.float32
F16 = mybir.dt.float16
AF = mybir.ActivationFunctionType

LAST_EXEC_NS = None


def _install_ntff_hook():
    try:
        import antenv.axon_hooks  # noqa: F401
        return True
    except ImportError:
        pass
    try:
        if "/root/.axon_site" not in sys.path:
            sys.path.insert(0, "/root/.axon_site")
        from trn_agent_boot.trn_boot import _ntff_profile_via_ctypes
        hook = _ntff_profile_via_ctypes("/opt/axon/libaxon_pjrt.so")
        if hook is None:
            return False
        import antenv
        mod = types.ModuleType("antenv.axon_hooks")
        mod._hook = hook
        mod.get_axon_ntff_profile_hook = lambda: mod._hook
        mod.set_axon_ntff_profile_hook = lambda h: setattr(mod, "_hook", h)
        antenv.axon_hooks = mod
        sys.modules["antenv.axon_hooks"] = mod
        return True
    except Exception:
        return False


def build_program(steps=T):
    nc = bacc.Bacc("TRN2", target_bir_lowering=False, debug=False)

    xT_d = nc.dram_tensor("xT", [128, KI, BSH], F16, kind="ExternalInput").ap()
    winT_d = nc.dram_tensor("winT", [128, KI, HID], F16, kind="ExternalInput").ap()
    wcT_d = nc.dram_tensor("wcT", [128, KH, 4 * HID], F16, kind="ExternalInput").ap()
    woutT_d = nc.dram_tensor("woutT", [128, KH, OUT_DIM], F16, kind="ExternalInput").ap()
    bin_d = nc.dram_tensor("bin", [128, KH], F32, kind="ExternalInput").ap()
    bc_d = nc.dram_tensor("bc", [128, NMT], F32, kind="ExternalInput").ap()
    bout_d = nc.dram_tensor("bout", [128, OUT_DIM], F32, kind="ExternalInput").ap()
    out_d = nc.dram_tensor("out", [BSH, steps, OUT_DIM], F32, kind="ExternalOutput").ap()
    # [p, m, t, o]: batch row = m*128 + p
    out_v = out_d.rearrange("(m p) t o -> p m t o", p=128)

    from contextlib import ExitStack
    with tile.TileContext(nc) as tc, ExitStack() as ctx:
        consts = ctx.enter_context(tc.tile_pool(name="consts", bufs=1))
        hpool = ctx.enter_context(tc.tile_pool(name="h", bufs=4))
        cpool = ctx.enter_context(tc.tile_pool(name="c", bufs=3))
        actp = ctx.enter_context(tc.tile_pool(name="acts", bufs=6))
        dvep = ctx.enter_context(tc.tile_pool(name="dvet", bufs=4))
        osbp = ctx.enter_context(tc.tile_pool(name="osb", bufs=4))
        pg_pool = ctx.enter_context(tc.tile_pool(name="pgates", bufs=3, space="PSUM"))
        po_pool = ctx.enter_context(tc.tile_pool(name="pout", bufs=2, space="PSUM"))

        # ---- load constants ----
        xT = consts.tile([128, KI, BSH], F16)
        nc.sync.dma_start(xT[:], xT_d[:])
        winT = consts.tile([128, KI, HID], F16)
        nc.sync.dma_start(winT[:], winT_d[:])
        wcT = consts.tile([128, KH, 4 * HID], F16)
        nc.sync.dma_start(wcT[:], wcT_d[:])
        woutT = consts.tile([128, KH, OUT_DIM], F16)
        nc.sync.dma_start(woutT[:], woutT_d[:])
        bin_sb = consts.tile([128, KH], F32)
        nc.sync.dma_start(bin_sb[:], bin_d[:])
        bc_sb = consts.tile([128, NMT], F32)
        nc.sync.dma_start(bc_sb[:], bc_d[:])
        bout_sb = consts.tile([128, OUT_DIM], F32)
        nc.sync.dma_start(bout_sb[:], bout_d[:])

        # ---- input projection: h0.T = relu(W_in @ x.T + b_in) ----
        h = hpool.tile([128, KH, BSH], F16, tag="h")
        ph0 = pg_pool.tile([128, 4, BSH], F32, tag="pg")
        for m in range(KH):
            for k in range(KI):
                nc.tensor.matmul(
                    ph0[:, m, :],
                    winT[:, k, m * 128:(m + 1) * 128],
                    xT[:, k, :],
                    start=(k == 0 and m % 2 == 0),
                    stop=(k == KI - 1 and m % 2 == 1),
                )
        for m in range(KH):
            nc.scalar.activation(
                h[:, m, :], ph0[:, m, :], AF.Relu, bias=bin_sb[:, m:m + 1]
            )

        bout_b = bout_sb[:].unsqueeze(1).broadcast_to([128, 2, OUT_DIM])

        def emit_outproj_mms(po, h_src, ks):
            # po is one PSUM bank: a single accumulation group (start on first
            # matmul touching the bank, stop on the last; unwritten bytes of a
            # started zero-region read as 0, so m=1's k=0 matmul may accumulate)
            for k in ks:
                for m in range(2):
                    nc.tensor.matmul(
                        po[:, m, :],
                        h_src[:, k, m * 128:(m + 1) * 128],
                        woutT[:, k, :],
                        start=(k == 0 and m == 0),
                        stop=(k == KH - 1 and m == 1),
                    )

        def emit_outproj_tail(po, t_idx):
            osb = osbp.tile([128, 2, OUT_DIM], F32, tag="osb", name=f"osb{t_idx}")
            nc.vector.tensor_add(osb[:], po[:], bout_b)
            nc.sync.dma_start(out_v[:, :, t_idx, :], osb[:])

        def emit_outproj(h_src, t_idx):
            po = po_pool.tile([128, 2, OUT_DIM], F32, tag="po", name=f"po{t_idx}")
            emit_outproj_mms(po, h_src, range(KH))
            emit_outproj_tail(po, t_idx)

        c = None
        for t in range(steps):
            h_new = hpool.tile([128, KH, BSH], F16, tag="h")
            c_new = cpool.tile([128, KH, BSH], F16, tag="c")
            wave = [None] * KH  # per wave: (sig_i, sig_f, sig_o, tng)
            pgs = [None] * KH

            def mm_block(j, ks):
                # pg spans 2 banks (gi 0,1 | gi 2,3): one accumulation group
                # per bank — start on the bank's first matmul, stop on its last
                for k in ks:
                    for gi in range(4):
                        mt = 4 * j + gi
                        nc.tensor.matmul(
                            pgs[j][:, gi, :],
                            wcT[:, k, mt * 128:(mt + 1) * 128],
                            h[:, k, :],
                            start=(k == 0 and gi % 2 == 0),
                            stop=(k == KH - 1 and gi % 2 == 1),
                        )

            def act_block(j):
                pg = pgs[j]
                sig_i = actp.tile([128, BSH], F16, tag="sig_i", name=f"si{t}_{j}")
                sig_f = actp.tile([128, BSH], F16, tag="sig_f", name=f"sf{t}_{j}")
                sig_o = actp.tile([128, BSH], F16, tag="sig_o", name=f"so{t}_{j}")
                tng = actp.tile([128, BSH], F16, tag="tng", name=f"tg{t}_{j}")
                nc.scalar.activation(sig_i[:], pg[:, 0, :], AF.Sigmoid, bias=bc_sb[:, 4 * j + 0:4 * j + 1])
                nc.scalar.activation(sig_f[:], pg[:, 1, :], AF.Sigmoid, bias=bc_sb[:, 4 * j + 1:4 * j + 2])
                nc.scalar.activation(tng[:], pg[:, 3, :], AF.Tanh, bias=bc_sb[:, 4 * j + 3:4 * j + 4])
                nc.scalar.activation(sig_o[:], pg[:, 2, :], AF.Sigmoid, bias=bc_sb[:, 4 * j + 2:4 * j + 3])
                wave[j] = (sig_i, sig_f, sig_o, tng)

            def cell_update(j):
                # c_new[j] = sig(f)*c[j] + sig(i)*tanh(g)
                sig_i, sig_f, sig_o, tng = wave[j]
                if t == 0:
                    # c0 == 0: c1 = sig(i) * tanh(g)
                    nc.vector.tensor_mul(c_new[:, j, :], sig_i[:], tng[:])
                else:
                    t1 = dvep.tile([128, BSH], F16, tag="t1", name=f"t1_{t}_{j}")
                    nc.vector.tensor_mul(t1[:], sig_i[:], tng[:])
                    nc.vector.tensor_mul(c_new[:, j, :], sig_f[:], c[:, j, :])
                    nc.vector.tensor_add(c_new[:, j, :], c_new[:, j, :], t1[:])

            def h_update(jpair):
                # tanh(c) merged over a wave pair, then h = sig(o) * tanh(c)
                tnc = dvep.tile([128, 2, BSH], F16, tag="tnc", name=f"tnc{t}_{jpair}")
                nc.scalar.activation(tnc[:], c_new[:, 2 * jpair:2 * jpair + 2, :], AF.Tanh)
                for j in (2 * jpair, 2 * jpair + 1):
                    nc.vector.tensor_mul(h_new[:, j, :], wave[j][2][:], tnc[:, j - 2 * jpair, :])

            def h_update_single(j, split=False):
                # last waves get individual tanh(c) so h[2] exits the
                # end-of-step dependency ring before h[3]'s chain completes;
                # h[3]'s multiply is split into batch halves so the next
                # step's k3 matmuls can start on the first half early
                tnc = dvep.tile([128, 2, BSH], F16, tag="tnc", name=f"tncs{t}_{j}")
                nc.scalar.activation(tnc[:, 0, :], c_new[:, j, :], AF.Tanh)
                if split:
                    hb = BSH // 2
                    nc.vector.tensor_mul(h_new[:, j, :hb], wave[j][2][:, :hb], tnc[:, 0, :hb])
                    nc.vector.tensor_mul(h_new[:, j, hb:], wave[j][2][:, hb:], tnc[:, 0, hb:])
                else:
                    nc.vector.tensor_mul(h_new[:, j, :], wave[j][2][:], tnc[:, 0, :])

            # PE stream: front-load k0-k2 matmuls of wave0/outproj/wave1 (they
            # need only early h slices) so the PE advances through the
            # end-of-step ring while the last wave's pointwise chain finishes;
            # the k3 blocks (gated on the final h slice) come right after.
            pgs[0] = pg_pool.tile([128, 4, BSH], F32, tag="pg", name=f"pg{t}_0")
            mm_block(0, range(KH - 1))
            po = None
            if t >= 1:
                po = po_pool.tile([128, 2, OUT_DIM], F32, tag="po", name=f"po{t-1}")
                emit_outproj_mms(po, h, range(KH - 1))
            pgs[1] = pg_pool.tile([128, 4, BSH], F32, tag="pg", name=f"pg{t}_1")
            mm_block(1, range(KH - 1))
            # wave0's k3 matmuls run in batch halves: the a-half issues as soon
            # as the first half of h[3] lands, overlapping the rest of its chain
            for half in range(2):
                lo, hi = half * (BSH // 2), (half + 1) * (BSH // 2)
                for gi in range(4):
                    mt = 4 * 0 + gi
                    nc.tensor.matmul(
                        pgs[0][:, gi, lo:hi],
                        wcT[:, KH - 1, mt * 128:(mt + 1) * 128],
                        h[:, KH - 1, lo:hi],
                        start=False,
                        stop=(half == 1 and gi % 2 == 1),
                    )
            act_block(0)
            mm_block(1, [KH - 1])
            act_block(1)
            cell_update(0)
            if t >= 1:
                emit_outproj_mms(po, h, [KH - 1])
                emit_outproj_tail(po, t - 1)
            pgs[2] = pg_pool.tile([128, 4, BSH], F32, tag="pg", name=f"pg{t}_2")
            mm_block(2, range(KH))
            act_block(2)
            cell_update(1)
            h_update(0)
            pgs[3] = pg_pool.tile([128, 4, BSH], F32, tag="pg", name=f"pg{t}_3")
            mm_block(3, range(KH))
            act_block(3)
            cell_update(2)
            h_update_single(2)
            cell_update(KH - 1)
            h_update_single(3, split=True)

            h = h_new
            c = c_new
        emit_outproj(h, steps - 1)

    nc.compile()
    return nc


_PROGRAM = None


def _get_program():
    global _PROGRAM
    if _PROGRAM is None:
        _PROGRAM = build_program()
    return _PROGRAM


def _pack_inputs(x, W_in, b_in, W_ih, b_ih, W_hh, b_hh, W_out, b_out):
    f16, f32 = np.float16, np.float32
    Wc = (np.asarray(W_ih, f32) + np.asarray(W_hh, f32))
    bc = (np.asarray(b_ih, f32) + np.asarray(b_hh, f32))
    # reorder gate rows to m-tiles [i_j, f_j, o_j, g_j] (PyTorch order i,f,g,o)
    base = {0: 0, 1: HID, 2: 3 * HID, 3: 2 * HID}  # gi -> original row block
    perm = np.concatenate([
        np.arange(base[gi] + j * 128, base[gi] + j * 128 + 128)
        for j in range(KH) for gi in range(4)
    ])
    Wc_r = Wc[perm]                      # [2048, 512]
    bc_r = bc[perm]                      # [2048]
    wcT = np.ascontiguousarray(
        Wc_r.T.reshape(KH, 128, 4 * HID).transpose(1, 0, 2).astype(f16))
    winT = np.ascontiguousarray(
        np.asarray(W_in, f32).T.reshape(KI, 128, HID).transpose(1, 0, 2).astype(f16))
    woutT = np.ascontiguousarray(
        np.asarray(W_out, f32).T.reshape(KH, 128, OUT_DIM).transpose(1, 0, 2).astype(f16))
    bin_p = np.ascontiguousarray(np.asarray(b_in, f32).reshape(KH, 128).T)
    bc_p = np.ascontiguousarray(bc_r.reshape(NMT, 128).T)
    bout_p = np.ascontiguousarray(np.broadcast_to(np.asarray(b_out, f32), (128, OUT_DIM)))

    shared = {
        "winT": winT, "wcT": wcT, "woutT": woutT,
        "bin": bin_p, "bc": bc_p, "bout": bout_p,
    }
    in_maps = []
    x = np.asarray(x, f32)
    for cid in range(NCORES):
        xs = x[cid * BSH:(cid + 1) * BSH]          # [256, 1024]
        xT = np.ascontiguousarray(
            xs.T.reshape(KI, 128, BSH).transpose(1, 0, 2).astype(f16))
        in_maps.append({"xT": xT, **shared})
    return in_maps


def kernel(x, W_in, b_in, W_ih, b_ih, W_hh, b_hh, W_out, b_out, trace=False):
    global LAST_EXEC_NS
    nc = _get_program()
    in_maps = _pack_inputs(x, W_in, b_in, W_ih, b_ih, W_hh, b_hh, W_out, b_out)
    if trace:
        trace = _install_ntff_hook()
    res = run_bass_kernel_spmd(nc, in_maps, core_ids=list(range(NCORES)), trace=trace)
    LAST_EXEC_NS = res.exec_time_ns
    return np.concatenate([res.results[c]["out"] for c in range(NCORES)], axis=0)


# revision 27
# speedup vs baseline: 1.0970x; 1.0011x over previous
"""Trainium2 Bass kernel for nn_LstmClassifier: batch-sharded LSTM over 8 cores.

Reference math (per batch row):
    h0 = relu(x @ W_in.T + b_in); c0 = 0
    64 steps of: gates = h @ (W_ih + W_hh).T + (b_ih + b_hh)   # input == hidden
                 i,f,g,o = split(gates); c = sig(f)*c + sig(i)*tanh(g); h = sig(o)*tanh(c)
    out[:, t, :] = h_t @ W_out.T + b_out

Device layout is hidden-major ("transposed"): h.T packed as [128 part, 4, 256],
so gate-unit biases are per-partition and fuse into the activation instructions,
and no transposes are needed anywhere in the recurrence.
"""
import sys
import types
import numpy as np

sys.path.insert(0, "/opt/trn_rl_repo")

import concourse.bass as bass  # noqa: E402
import concourse.tile as tile  # noqa: E402
from concourse import bacc, mybir  # noqa: E402
from concourse.bass_utils import run_bass_kernel_spmd  # noqa: E402

B, IN_DIM, HID, OUT_DIM, T = 2048, 1024, 512, 256, 64
NCORES = 8
BSH = B // NCORES          # 256 batch rows per core
KH = HID // 128            # 4 hidden k-tiles
KI = IN_DIM // 128         # 8 input k-tiles
NMT = 4 * HID // 128       # 16 gate m-tiles
F32 = mybir.dt.float32
F16 = mybir.dt.float16
AF = mybir.ActivationFunctionType

LAST_EXEC_NS = None


def _install_ntff_hook():
    try:
        import antenv.axon_hooks  # noqa: F401
        return True
    except ImportError:
        pass
    try:
        if "/root/.axon_site" not in sys.path:
            sys.path.insert(0, "/root/.axon_site")
        from trn_agent_boot.trn_boot import _ntff_profile_via_ctypes
        hook = _ntff_profile_via_ctypes("/opt/axon/libaxon_pjrt.so")
        if hook is None:
            return False
        import antenv
        mod = types.ModuleType("antenv.axon_hooks")
        mod._hook = hook
        mod.get_axon_ntff_profile_hook = lambda: mod._hook
        mod.set_axon_ntff_profile_hook = lambda h: setattr(mod, "_hook", h)
        antenv.axon_hooks = mod
        sys.modules["antenv.axon_hooks"] = mod
        return True
    except Exception:
        return False


def build_program(steps=T):
    nc = bacc.Bacc("TRN2", target_bir_lowering=False, debug=False)

    xT_d = nc.dram_tensor("xT", [128, KI, BSH], F16, kind="ExternalInput").ap()
    winT_d = nc.dram_tensor("winT", [128, KI, HID], F16, kind="ExternalInput").ap()
    wcT_d = nc.dram_tensor("wcT", [128, KH, 4 * HID], F16, kind="ExternalInput").ap()
    woutT_d = nc.dram_tensor("woutT", [128, KH, OUT_DIM], F16, kind="ExternalInput").ap()
    bin_d = nc.dram_tensor("bin", [128, KH], F32, kind="ExternalInput").ap()
    bc_d = nc.dram_tensor("bc", [128, NMT], F32, kind="ExternalInput").ap()
    bout_d = nc.dram_tensor("bout", [128, OUT_DIM], F32, kind="ExternalInput").ap()
    out_d = nc.dram_tensor("out", [BSH, steps, OUT_DIM], F32, kind="ExternalOutput").ap()
    # [p, m, t, o]: batch row = m*128 + p
    out_v = out_d.rearrange("(m p) t o -> p m t o", p=128)

    from contextlib import ExitStack
    with tile.TileContext(nc) as tc, ExitStack() as ctx:
        consts = ctx.enter_context(tc.tile_pool(name="consts", bufs=1))
        hpool = ctx.enter_context(tc.tile_pool(name="h", bufs=4))
        cpool = ctx.enter_context(tc.tile_pool(name="c", bufs=3))
        actp = ctx.enter_context(tc.tile_pool(name="acts", bufs=6))
        dvep = ctx.enter_context(tc.tile_pool(name="dvet", bufs=4))
        osbp = ctx.enter_context(tc.tile_pool(name="osb", bufs=4))
        pg_pool = ctx.enter_context(tc.tile_pool(name="pgates", bufs=3, space="PSUM"))
        po_pool = ctx.enter_context(tc.tile_pool(name="pout", bufs=2, space="PSUM"))

        # ---- load constants ----
        xT = consts.tile([128, KI, BSH], F16)
        nc.sync.dma_start(xT[:], xT_d[:])
        winT = consts.tile([128, KI, HID], F16)
        nc.sync.dma_start(winT[:], winT_d[:])
        wcT = consts.tile([128, KH, 4 * HID], F16)
        nc.sync.dma_start(wcT[:], wcT_d[:])
        woutT = consts.tile([128, KH, OUT_DIM], F16)
        nc.sync.dma_start(woutT[:], woutT_d[:])
        bin_sb = consts.tile([128, KH], F32)
        nc.sync.dma_start(bin_sb[:], bin_d[:])
        bc_sb = consts.tile([128, NMT], F32)
        nc.sync.dma_start(bc_sb[:], bc_d[:])
        bout_sb = consts.tile([128, OUT_DIM], F32)
        nc.sync.dma_start(bout_sb[:], bout_d[:])

        # ---- input projection: h0.T = relu(W_in @ x.T + b_in) ----
        h = hpool.tile([128, KH, BSH], F16, tag="h")
        ph0 = pg_pool.tile([128, 4, BSH], F32, tag="pg")
        for m in range(KH):
            for k in range(KI):
                nc.tensor.matmul(
                    ph0[:, m, :],
                    winT[:, k, m * 128:(m + 1) * 128],
                    xT[:, k, :],
                    start=(k == 0 and m % 2 == 0),
                    stop=(k == KI - 1 and m % 2 == 1),
                )
        for m in range(KH):
            nc.scalar.activation(
                h[:, m, :], ph0[:, m, :], AF.Relu, bias=bin_sb[:, m:m + 1]
            )

        bout_b = bout_sb[:].unsqueeze(1).broadcast_to([128, 2, OUT_DIM])

        def emit_outproj_mms(po, h_src, ks):
            # po is one PSUM bank: a single accumulation group (start on first
            # matmul touching the bank, stop on the last; unwritten bytes of a
            # started zero-region read as 0, so m=1's k=0 matmul may accumulate)
            for k in ks:
                for m in range(2):
                    nc.tensor.matmul(
                        po[:, m, :],
                        h_src[:, k, m * 128:(m + 1) * 128],
                        woutT[:, k, :],
                        start=(k == 0 and m == 0),
                        stop=(k == KH - 1 and m == 1),
                    )

        def emit_outproj_tail(po, t_idx):
            osb = osbp.tile([128, 2, OUT_DIM], F32, tag="osb", name=f"osb{t_idx}")
            nc.vector.tensor_add(osb[:], po[:], bout_b)
            nc.sync.dma_start(out_v[:, :, t_idx, :], osb[:])

        def emit_outproj(h_src, t_idx):
            po = po_pool.tile([128, 2, OUT_DIM], F32, tag="po", name=f"po{t_idx}")
            emit_outproj_mms(po, h_src, range(KH))
            emit_outproj_tail(po, t_idx)

        c = None
        for t in range(steps):
            h_new = hpool.tile([128, KH, BSH], F16, tag="h")
            c_new = cpool.tile([128, KH, BSH], F16, tag="c")
            wave = [None] * KH  # per wave: (sig_i, sig_f, sig_o, tng)
            pgs = [None] * KH

            def mm_block(j, ks):
                # pg spans 2 banks (gi 0,1 | gi 2,3): one accumulation group
                # per bank — start on the bank's first matmul, stop on its last
                for k in ks:
                    for gi in range(4):
                        mt = 4 * j + gi
                        nc.tensor.matmul(
                            pgs[j][:, gi, :],
                            wcT[:, k, mt * 128:(mt + 1) * 128],
                            h[:, k, :],
                            start=(k == 0 and gi % 2 == 0),
                            stop=(k == KH - 1 and gi % 2 == 1),
                        )

            def act_block(j):
                pg = pgs[j]
                sig_i = actp.tile([128, BSH], F16, tag="sig_i", name=f"si{t}_{j}")
                sig_f = actp.tile([128, BSH], F16, tag="sig_f", name=f"sf{t}_{j}")
                sig_o = actp.tile([128, BSH], F16, tag="sig_o", name=f"so{t}_{j}")
                tng = actp.tile([128, BSH], F16, tag="tng", name=f"tg{t}_{j}")
                nc.scalar.activation(sig_i[:], pg[:, 0, :], AF.Sigmoid, bias=bc_sb[:, 4 * j + 0:4 * j + 1])
                nc.scalar.activation(sig_f[:], pg[:, 1, :], AF.Sigmoid, bias=bc_sb[:, 4 * j + 1:4 * j + 2])
                nc.scalar.activation(tng[:], pg[:, 3, :], AF.Tanh, bias=bc_sb[:, 4 * j + 3:4 * j + 4])
                nc.scalar.activation(sig_o[:], pg[:, 2, :], AF.Sigmoid, bias=bc_sb[:, 4 * j + 2:4 * j + 3])
                wave[j] = (sig_i, sig_f, sig_o, tng)

            def cell_update(j):
                # c_new[j] = sig(f)*c[j] + sig(i)*tanh(g)
                sig_i, sig_f, sig_o, tng = wave[j]
                if t == 0:
                    # c0 == 0: c1 = sig(i) * tanh(g)
                    nc.vector.tensor_mul(c_new[:, j, :], sig_i[:], tng[:])
                else:
                    t1 = dvep.tile([128, BSH], F16, tag="t1", name=f"t1_{t}_{j}")
                    nc.vector.tensor_mul(t1[:], sig_i[:], tng[:])
                    nc.vector.tensor_mul(c_new[:, j, :], sig_f[:], c[:, j, :])
                    nc.vector.tensor_add(c_new[:, j, :], c_new[:, j, :], t1[:])

            def h_update(jpair):
                # tanh(c) merged over a wave pair, then h = sig(o) * tanh(c)
                tnc = dvep.tile([128, 2, BSH], F16, tag="tnc", name=f"tnc{t}_{jpair}")
                nc.scalar.activation(tnc[:], c_new[:, 2 * jpair:2 * jpair + 2, :], AF.Tanh)
                for j in (2 * jpair, 2 * jpair + 1):
                    nc.vector.tensor_mul(h_new[:, j, :], wave[j][2][:], tnc[:, j - 2 * jpair, :])

            def h_update_single(j, split=False):
                # last waves get individual tanh(c) so h[2] exits the
                # end-of-step dependency ring before h[3]'s chain completes;
                # h[3]'s multiply is split into batch halves so the next
                # step's k3 matmuls can start on the first half early
                tnc = dvep.tile([128, 2, BSH], F16, tag="tnc", name=f"tncs{t}_{j}")
                nc.scalar.activation(tnc[:, 0, :], c_new[:, j, :], AF.Tanh)
                if split:
                    hb = BSH // 2
                    nc.vector.tensor_mul(h_new[:, j, :hb], wave[j][2][:, :hb], tnc[:, 0, :hb])
                    nc.vector.tensor_mul(h_new[:, j, hb:], wave[j][2][:, hb:], tnc[:, 0, hb:])
                else:
                    nc.vector.tensor_mul(h_new[:, j, :], wave[j][2][:], tnc[:, 0, :])

            # PE stream: front-load k0-k2 matmuls of wave0/outproj/wave1 (they
            # need only early h slices) so the PE advances through the
            # end-of-step ring while the last wave's pointwise chain finishes;
            # the k3 blocks (gated on the final h slice) come right after.
            pgs[0] = pg_pool.tile([128, 4, BSH], F32, tag="pg", name=f"pg{t}_0")
            mm_block(0, range(KH - 1))
            po = None
            if t >= 1:
                po = po_pool.tile([128, 2, OUT_DIM], F32, tag="po", name=f"po{t-1}")
                emit_outproj_mms(po, h, range(KH - 1))
            pgs[1] = pg_pool.tile([128, 4, BSH], F32, tag="pg", name=f"pg{t}_1")
            mm_block(1, range(KH - 1))
            # wave0's k3 matmuls run in batch halves: the a-half issues as soon
            # as the first half of h[3] lands, overlapping the rest of its chain
            # bank A (i,f — gating the first sigmoid of the ring) completes
            # before any bank B matmul issues
            for pair in range(2):
                for half in range(2):
                    lo, hi = half * (BSH // 2), (half + 1) * (BSH // 2)
                    for gi in (2 * pair, 2 * pair + 1):
                        mt = 4 * 0 + gi
                        nc.tensor.matmul(
                            pgs[0][:, gi, lo:hi],
                            wcT[:, KH - 1, mt * 128:(mt + 1) * 128],
                            h[:, KH - 1, lo:hi],
                            start=False,
                            stop=(half == 1 and gi % 2 == 1),
                        )
            act_block(0)
            mm_block(1, [KH - 1])
            act_block(1)
            cell_update(0)
            if t >= 1:
                emit_outproj_mms(po, h, [KH - 1])
                emit_outproj_tail(po, t - 1)
            pgs[2] = pg_pool.tile([128, 4, BSH], F32, tag="pg", name=f"pg{t}_2")
            mm_block(2, range(KH))
            act_block(2)
            cell_update(1)
            h_update(0)
            pgs[3] = pg_pool.tile([128, 4, BSH], F32, tag="pg", name=f"pg{t}_3")
            mm_block(3, range(KH))
            act_block(3)
            cell_update(2)
            h_update_single(2)
            cell_update(KH - 1)
            h_update_single(3, split=True)

            h = h_new
            c = c_new
        emit_outproj(h, steps - 1)

    nc.compile()
    return nc


_PROGRAM = None


def _get_program():
    global _PROGRAM
    if _PROGRAM is None:
        _PROGRAM = build_program()
    return _PROGRAM


def _pack_inputs(x, W_in, b_in, W_ih, b_ih, W_hh, b_hh, W_out, b_out):
    f16, f32 = np.float16, np.float32
    Wc = (np.asarray(W_ih, f32) + np.asarray(W_hh, f32))
    bc = (np.asarray(b_ih, f32) + np.asarray(b_hh, f32))
    # reorder gate rows to m-tiles [i_j, f_j, o_j, g_j] (PyTorch order i,f,g,o)
    base = {0: 0, 1: HID, 2: 3 * HID, 3: 2 * HID}  # gi -> original row block
    perm = np.concatenate([
        np.arange(base[gi] + j * 128, base[gi] + j * 128 + 128)
        for j in range(KH) for gi in range(4)
    ])
    Wc_r = Wc[perm]                      # [2048, 512]
    bc_r = bc[perm]                      # [2048]
    wcT = np.ascontiguousarray(
        Wc_r.T.reshape(KH, 128, 4 * HID).transpose(1, 0, 2).astype(f16))
    winT = np.ascontiguousarray(
        np.asarray(W_in, f32).T.reshape(KI, 128, HID).transpose(1, 0, 2).astype(f16))
    woutT = np.ascontiguousarray(
        np.asarray(W_out, f32).T.reshape(KH, 128, OUT_DIM).transpose(1, 0, 2).astype(f16))
    bin_p = np.ascontiguousarray(np.asarray(b_in, f32).reshape(KH, 128).T)
    bc_p = np.ascontiguousarray(bc_r.reshape(NMT, 128).T)
    bout_p = np.ascontiguousarray(np.broadcast_to(np.asarray(b_out, f32), (128, OUT_DIM)))

    shared = {
        "winT": winT, "wcT": wcT, "woutT": woutT,
        "bin": bin_p, "bc": bc_p, "bout": bout_p,
    }
    in_maps = []
    x = np.asarray(x, f32)
    for cid in range(NCORES):
        xs = x[cid * BSH:(cid + 1) * BSH]          # [256, 1024]
        xT = np.ascontiguousarray(
            xs.T.reshape(KI, 128, BSH).transpose(1, 0, 2).astype(f16))
        in_maps.append({"xT": xT, **shared})
    return in_maps


def kernel(x, W_in, b_in, W_ih, b_ih, W_hh, b_hh, W_out, b_out, trace=False):
    global LAST_EXEC_NS
    nc = _get_program()
    in_maps = _pack_inputs(x, W_in, b_in, W_ih, b_ih, W_hh, b_hh, W_out, b_out)
    if trace:
        trace = _install_ntff_hook()
    res = run_bass_kernel_spmd(nc, in_maps, core_ids=list(range(NCORES)), trace=trace)
    LAST_EXEC_NS = res.exec_time_ns
    return np.concatenate([res.results[c]["out"] for c in range(NCORES)], axis=0)
